# revision 9
# baseline (speedup 1.0000x reference)
"""ClusterGCNConvNet on 8 TRN2 NeuronCores.

Sharding: nodes partitioned into 8 contiguous ranges (12500/core). Each core
aggregates over the edges whose dst lands in its range, gathering source rows
with dma_gather (int16 indices -> 4 source chunks of 25000 rows). Layer
boundaries exchange the transformed activations y = h @ W_out with an
AllGather, so the per-layer gather reads from a replicated full-size buffer.

Per (dst-block of 128, chunk) the edge segment is padded to a multiple of 128
(same padded length on every core so all 8 cores share one Bass program);
segment-sum is a one-hot matmul: S[e, d] = (dstoff[e] == d) built with an
iota/is_equal, PSUM-accumulated over the segment, then added into an SBUF
accumulator. Transforms run in transposed space (h^T resident in SBUF).
"""
import sys, time
sys.path.insert(0, "/opt/trn_rl_repo")
import numpy as np

N, E = 100_000, 1_600_000
D = 64
NCORES = 8
NP = N // NCORES          # 12500 nodes per core
P = 128
NB = (NP + P - 1) // P    # 98 dst blocks per core
NPAD = NB * P             # 12544
CH = 25_000               # gather-source chunk rows (int16 index range)
NCH = 4
NGRP = 14                 # block groups per chunk pass
GBS = NB // NGRP          # 14 blocks per group

_STATE = {}


class _SpmdRunner:
    """Execute a prebuilt Bass module on 8 cores via PJRT (jit once, run many)."""

    def __init__(self, nc, n_cores):
        import jax
        import jax.numpy as jnp
        from jax.experimental.shard_map import shard_map
        from jax.sharding import Mesh, PartitionSpec
        import concourse.mybir as mybir
        from concourse.bass2jax import (_bass_exec_p, install_neuronx_cc_hook,
                                        partition_id_tensor)
        install_neuronx_cc_hook()
        self.jax, self.nc, self.n_cores = jax, nc, n_cores
        partition_name = nc.partition_id_tensor.name if nc.partition_id_tensor else None
        in_names, out_names, out_avals, zero_outs = [], [], [], []
        for alloc in nc.m.functions[0].allocations:
            if not isinstance(alloc, mybir.MemoryLocationSet):
                continue
            name = alloc.memorylocations[0].name
            if alloc.kind == "ExternalInput":
                if name != partition_name and (nc.dbg_addr is None or name != nc.dbg_addr.name):
                    in_names.append(name)
            elif alloc.kind == "ExternalOutput":
                shape = tuple(alloc.tensor_shape)
                dtype = mybir.dt.np(alloc.dtype)
                out_names.append(name)
                out_avals.append(jax.core.ShapedArray(shape, dtype))
                zero_outs.append(np.zeros(shape, dtype))
        self.in_names, self.out_names = in_names, out_names
        self.out_avals, self.zero_outs = out_avals, zero_outs
        n_params, n_outs = len(in_names), len(out_names)
        all_in_names = list(in_names) + list(out_names)
        dbg_name = nc.dbg_addr.name if nc.dbg_addr is not None else None
        if dbg_name is not None:
            all_in_names.append(dbg_name)
        if partition_name is not None:
            all_in_names.append(partition_name)

        def _body(*args):
            operands = list(args)
            if dbg_name is not None:
                operands.append(jnp.zeros((1, 2), jnp.uint32))
            if partition_name is not None:
                operands.append(partition_id_tensor())
            return tuple(_bass_exec_p.bind(
                *operands, out_avals=tuple(out_avals),
                in_names=tuple(all_in_names), out_names=tuple(out_names),
                lowering_input_output_aliases=(),
                sim_require_finite=False, sim_require_nnan=False, nc=nc))

        devices = jax.devices()[:n_cores]
        self.mesh = Mesh(np.asarray(devices), ("core",))
        self.PartitionSpec = PartitionSpec
        in_specs = (PartitionSpec("core"),) * (n_params + n_outs)
        out_specs = (PartitionSpec("core"),) * n_outs
        donate = tuple(range(n_params, n_params + n_outs))
        self.jitted = jax.jit(
            shard_map(_body, mesh=self.mesh, in_specs=in_specs,
                      out_specs=out_specs, check_rep=False),
            donate_argnums=donate, keep_unused=True)

    def run(self, in_maps, repeats=1):
        jax, n = self.jax, self.n_cores
        from jax.sharding import NamedSharding
        shard = NamedSharding(self.mesh, self.PartitionSpec("core"))
        concat_in = [
            np.concatenate([np.asarray(in_maps[c][name]) for c in range(n)], axis=0)
            for name in self.in_names]
        dev_in = [jax.device_put(a, shard) for a in concat_in]
        times, out_arrs = [], None
        for _ in range(repeats):
            zeros = [np.zeros((n * z.shape[0], *z.shape[1:]), z.dtype)
                     for z in self.zero_outs]
            dev_zeros = [jax.device_put(a, shard) for a in zeros]
            for a in dev_zeros:
                a.block_until_ready()
            t0 = time.perf_counter()
            out_arrs = self.jitted(*dev_in, *dev_zeros)
            for a in out_arrs:
                a.block_until_ready()
            times.append(time.perf_counter() - t0)
        results = [
            {name: np.asarray(out_arrs[i]).reshape(n, *self.out_avals[i].shape)[c]
             for i, name in enumerate(self.out_names)}
            for c in range(n)]
        return results, times


def _build_meta(edge_index):
    src_g = np.asarray(edge_index[0]).astype(np.int64)
    dst_g = np.asarray(edge_index[1]).astype(np.int64)
    deg = np.bincount(dst_g, minlength=N).astype(np.float32)
    deginv_full = (1.0 / np.clip(deg, 1.0, None)).astype(np.float32)

    per_core = []
    counts = np.zeros((NCORES, NCH, NB), np.int64)
    for p in range(NCORES):
        m = (dst_g >= p * NP) & (dst_g < (p + 1) * NP)
        s, d = src_g[m], dst_g[m] - p * NP
        b = d // P
        c = s // CH
        order = np.argsort(c * NB + b, kind="stable")
        s, d, b, c = s[order], d[order], b[order], c[order]
        np.add.at(counts[p], (c, b), 1)
        per_core.append((s, d, b, c))

    ksub = np.maximum(1, -(-counts.max(axis=0) // P))  # [NCH, NB] tiles per run
    # tile layout order: for c, for g, for b in group -> ksub[c][b] tiles
    toff = np.zeros((NCH, NB), np.int64)
    t = 0
    for c in range(NCH):
        for g in range(NGRP):
            for b in range(g * GBS, (g + 1) * GBS):
                toff[c, b] = t
                t += ksub[c, b]
    T_tot = t
    R_tot = T_tot * P

    gidx_w = np.zeros((NCORES, 128, R_tot // 16), np.int16)
    dstoff = np.full((NCORES, 128, T_tot), 255.0, np.float32)
    for p in range(NCORES):
        s, d, b, c = per_core[p]
        flat_idx = np.zeros(R_tot, np.int16)
        flat_off = np.full(R_tot, 255.0, np.float32)
        pos = np.searchsorted(c * NB + b, np.arange(NCH * NB), side="left")
        pos = np.append(pos, len(s))
        for cc in range(NCH):
            for bb in range(NB):
                lo, hi = pos[cc * NB + bb], pos[cc * NB + bb + 1]
                t0 = toff[cc, bb] * P
                n = hi - lo
                flat_idx[t0:t0 + n] = (s[lo:hi] - cc * CH).astype(np.int16)
                flat_off[t0:t0 + n] = (d[lo:hi] % P).astype(np.float32)
        w = flat_idx.reshape(R_tot // 16, 16).T            # [16, R/16]
        gidx_w[p] = np.tile(w, (8, 1))
        dstoff[p] = flat_off.reshape(T_tot, P).T            # [128, T_tot]

    # per-instruction (c,g) row counts and offsets
    n_cg = np.zeros((NCH, NGRP), np.int64)
    for c in range(NCH):
        for g in range(NGRP):
            n_cg[c, g] = ksub[c, g * GBS:(g + 1) * GBS].sum() * P
    ioff = np.zeros((NCH, NGRP), np.int64)   # int16-column offset of instr
    acc = 0
    for c in range(NCH):
        for g in range(NGRP):
            ioff[c, g] = acc // 16
            acc += n_cg[c, g]

    deginv_pad = np.ones((NCORES, 128, NB), np.float32)
    for p in range(NCORES):
        dv = np.ones(NPAD, np.float32)
        dv[:NP] = deginv_full[p * NP:(p + 1) * NP]
        deginv_pad[p] = dv.reshape(NB, P).T
    return dict(ksub=ksub, toff=toff, T_tot=T_tot, R_tot=R_tot, n_cg=n_cg,
                ioff=ioff, gidx_w=gidx_w, dstoff=dstoff, deginv=deginv_pad)


def _build_program(meta):
    import concourse.bacc as bacc
    import concourse.bass as bass
    import concourse.mybir as mybir
    import concourse.tile as tile
    from concourse.masks import make_identity
    dt = mybir.dt
    ksub, toff = meta["ksub"], meta["toff"]
    T_tot, R_tot = meta["T_tot"], meta["R_tot"]
    n_cg, ioff = meta["n_cg"], meta["ioff"]

    nc = bacc.Bacc("TRN2", target_bir_lowering=False, debug=False,
                   num_devices=NCORES)
    x_full = nc.dram_tensor("x_full", [N, D], dt.float32, kind="ExternalInput")
    xT_in = nc.dram_tensor("xT", [D, NPAD], dt.float32, kind="ExternalInput")
    gidx = nc.dram_tensor("gidx", [128, R_tot // 16], dt.int16, kind="ExternalInput")
    dsto = nc.dram_tensor("dsto", [128, T_tot], dt.float32, kind="ExternalInput")
    dgin = nc.dram_tensor("dgin", [128, NB], dt.float32, kind="ExternalInput")
    w0 = nc.dram_tensor("w0", [D, D], dt.float32, kind="ExternalInput")
    wr0 = nc.dram_tensor("wr0", [D, D], dt.float32, kind="ExternalInput")
    w1 = nc.dram_tensor("w1", [D, D], dt.float32, kind="ExternalInput")
    wr1 = nc.dram_tensor("wr1", [D, D], dt.float32, kind="ExternalInput")
    w2p = nc.dram_tensor("w2p", [D, 16], dt.float32, kind="ExternalInput")
    wr2p = nc.dram_tensor("wr2p", [D, 16], dt.float32, kind="ExternalInput")
    b0_in = nc.dram_tensor("b0", [D, 1], dt.float32, kind="ExternalInput")
    b1_in = nc.dram_tensor("b1", [D, 1], dt.float32, kind="ExternalInput")
    b2_in = nc.dram_tensor("b2", [128, 16], dt.float32, kind="ExternalInput")
    out_ext = nc.dram_tensor("out", [NP, 10], dt.float32, kind="ExternalOutput")

    y2_full = nc.dram_tensor("y2_full", [N, D], dt.float32, kind="Internal",
                             addr_space="Shared")
    y3_full = nc.dram_tensor("y3_full", [N, 16], dt.float32, kind="Internal",
                             addr_space="Shared")
    y3_pad = nc.dram_tensor("y3_pad", [N, D], dt.float32, kind="Internal")

    with tile.TileContext(nc) as tc:
        with tc.tile_pool(name="res", bufs=1) as res, \
             tc.tile_pool(name="gpool", bufs=2) as gpool, \
             tc.tile_pool(name="spool", bufs=3) as spool, \
             tc.tile_pool(name="wk", bufs=3) as wk, \
             tc.tile_pool(name="ps_a", bufs=2, space="PSUM") as ps_a, \
             tc.tile_pool(name="ps_b", bufs=2, space="PSUM") as ps_b, \
             tc.tile_pool(name="dram", bufs=1, space="DRAM") as dram:

            it = res.tile([128, R_tot // 16], dt.int16)
            nc.sync.dma_start(out=it[:], in_=gidx[:, :])
            doff = res.tile([128, T_tot], dt.float32)
            nc.sync.dma_start(out=doff[:], in_=dsto[:, :])
            dgv = res.tile([128, NB], dt.float32)
            nc.sync.dma_start(out=dgv[:], in_=dgin[:, :])
            W0 = res.tile([D, D], dt.float32); nc.sync.dma_start(out=W0[:], in_=w0[:, :])
            Wr0 = res.tile([D, D], dt.float32); nc.sync.dma_start(out=Wr0[:], in_=wr0[:, :])
            W1 = res.tile([D, D], dt.float32); nc.sync.dma_start(out=W1[:], in_=w1[:, :])
            Wr1 = res.tile([D, D], dt.float32); nc.sync.dma_start(out=Wr1[:], in_=wr1[:, :])
            W2p = res.tile([D, 16], dt.float32); nc.sync.dma_start(out=W2p[:], in_=w2p[:, :])
            Wr2p = res.tile([D, 16], dt.float32); nc.sync.dma_start(out=Wr2p[:], in_=wr2p[:, :])
            B0 = res.tile([D, 1], dt.float32); nc.sync.dma_start(out=B0[:], in_=b0_in[:, :])
            B1 = res.tile([D, 1], dt.float32); nc.sync.dma_start(out=B1[:], in_=b1_in[:, :])
            B2 = res.tile([128, 16], dt.float32); nc.sync.dma_start(out=B2[:], in_=b2_in[:, :])

            ident = res.tile([128, 128], dt.float32)
            make_identity(nc, ident[:])
            iota_i = res.tile([128, 128], dt.int32)
            nc.gpsimd.iota(iota_i[:], pattern=[[1, 128]], base=0, channel_multiplier=0)
            iota_f = res.tile([128, 128], dt.float32)
            nc.vector.tensor_copy(out=iota_f[:], in_=iota_i[:])

            h1T = res.tile([D, NPAD], dt.float32)
            h2T = res.tile([D, NPAD], dt.float32)
            aggS = res.tile([128, NB * D], dt.float32)

            def segsum(src_ap):
                """gather + one-hot segment-sum into aggS (raw sums)."""
                for c in range(NCH):
                    for g in range(NGRP):
                        n = int(n_cg[c, g])
                        nk = n // P
                        gt = gpool.tile([128, nk, D], dt.float32, tag="g")
                        nc.gpsimd.dma_gather(
                            out_ap=gt[:], in_ap=src_ap[c * CH:(c + 1) * CH, :],
                            idxs_ap=it[:, int(ioff[c, g]):int(ioff[c, g]) + n // 16],
                            num_idxs=n, num_idxs_reg=n, elem_size=D,
                            single_packet=False)
                        base_t = int(toff[c, g * GBS])
                        for b in range(g * GBS, (g + 1) * GBS):
                            kk = int(ksub[c, b])
                            t0 = int(toff[c, b])
                            ps = ps_a.tile([128, D], dt.float32, tag="ps_a")
                            for k in range(kk):
                                S = spool.tile([128, 128], dt.float32, tag="S")
                                nc.vector.tensor_tensor(
                                    out=S[:],
                                    in0=doff[:, t0 + k:t0 + k + 1].to_broadcast([128, 128]),
                                    in1=iota_f[:],
                                    op=mybir.AluOpType.is_equal)
                                nc.tensor.matmul(
                                    ps[:], lhsT=S[:], rhs=gt[:, t0 - base_t + k, :],
                                    start=(k == 0), stop=(k == kk - 1))
                            dst = aggS[:, b * D:(b + 1) * D]
                            if c == 0:
                                nc.vector.tensor_copy(out=dst, in_=ps[:])
                            else:
                                nc.vector.tensor_add(out=dst, in0=dst, in1=ps[:])

            # ---------------- layer 1 ----------------
            cc1_in = dram.tile([NP, D], dt.float32)
            segsum(x_full[:, :])
            for b in range(NB):
                rows = min(P, NP - b * P)
                scaled = wk.tile([128, D], dt.float32, tag="scaled")
                nc.vector.tensor_scalar_mul(out=scaled[:], in0=aggS[:, b * D:(b + 1) * D],
                                            scalar1=dgv[:, b:b + 1])
                ps_t = ps_b.tile([D, 128], dt.float32, tag="ps_t")
                nc.tensor.transpose(out=ps_t[:], in_=scaled[:], identity=ident[:])
                aggT = wk.tile([D, 128], dt.float32, tag="aggT")
                nc.vector.tensor_copy(out=aggT[:], in_=ps_t[:])
                xTb = wk.tile([D, 128], dt.float32, tag="xTb")
                nc.sync.dma_start(out=xTb[:], in_=xT_in[:, b * P:(b + 1) * P])
                ps_h = ps_b.tile([D, 128], dt.float32, tag="ps_t")
                nc.tensor.matmul(ps_h[:], lhsT=W0[:], rhs=aggT[:], start=True, stop=False)
                nc.tensor.matmul(ps_h[:], lhsT=Wr0[:], rhs=xTb[:],
                                 start=False, stop=True)
                nc.scalar.activation(out=h1T[:, b * P:(b + 1) * P], in_=ps_h[:],
                                     func=mybir.ActivationFunctionType.Relu, bias=B0[:])
                ps_y = ps_a.tile([128, D], dt.float32, tag="ps_a")
                nc.tensor.matmul(ps_y[:], lhsT=h1T[:, b * P:(b + 1) * P], rhs=W1[:],
                                 start=True, stop=True)
                yrow = wk.tile([128, D], dt.float32, tag="yrow")
                nc.vector.tensor_copy(out=yrow[:], in_=ps_y[:])
                nc.sync.dma_start(out=cc1_in[b * P:b * P + rows, :], in_=yrow[:rows, :])
            nc.gpsimd.collective_compute(
                "AllGather", mybir.AluOpType.bypass,
                replica_groups=[list(range(NCORES))],
                ins=[cc1_in[:].opt()], outs=[y2_full[:, :].opt()])

            # ---------------- layer 2 ----------------
            cc2_in = dram.tile([NP, 16], dt.float32)
            segsum(y2_full[:, :])
            for b in range(NB):
                rows = min(P, NP - b * P)
                scaled = wk.tile([128, D], dt.float32, tag="scaled")
                nc.vector.tensor_scalar_mul(out=scaled[:], in0=aggS[:, b * D:(b + 1) * D],
                                            scalar1=dgv[:, b:b + 1])
                ps_t = ps_b.tile([D, 128], dt.float32, tag="ps_t")
                nc.tensor.transpose(out=ps_t[:], in_=scaled[:], identity=ident[:])
                tT = wk.tile([D, 128], dt.float32, tag="aggT")
                nc.vector.tensor_copy(out=tT[:], in_=ps_t[:])
                ps_r = ps_b.tile([D, 128], dt.float32, tag="ps_t")
                nc.tensor.matmul(ps_r[:], lhsT=Wr1[:], rhs=h1T[:, b * P:(b + 1) * P],
                                 start=True, stop=True)
                hsum = wk.tile([D, 128], dt.float32, tag="hsum")
                nc.vector.tensor_add(out=hsum[:], in0=tT[:], in1=ps_r[:])
                nc.scalar.activation(out=h2T[:, b * P:(b + 1) * P], in_=hsum[:],
                                     func=mybir.ActivationFunctionType.Relu, bias=B1[:])
                ps_y = ps_a.tile([128, 16], dt.float32, tag="ps_y3")
                nc.tensor.matmul(ps_y[:], lhsT=h2T[:, b * P:(b + 1) * P], rhs=W2p[:],
                                 start=True, stop=True)
                yrow = wk.tile([128, 16], dt.float32, tag="yrow3")
                nc.vector.tensor_copy(out=yrow[:], in_=ps_y[:])
                nc.sync.dma_start(out=cc2_in[b * P:b * P + rows, :], in_=yrow[:rows, :])
            nc.gpsimd.collective_compute(
                "AllGather", mybir.AluOpType.bypass,
                replica_groups=[list(range(NCORES))],
                ins=[cc2_in[:].opt()], outs=[y3_full[:, :].opt()])
            # expand [N,16] -> [N,64]-strided so gather rows are 256B apart
            for q in range(NCH):
                nc.sync.dma_start(out=y3_pad[q * CH:(q + 1) * CH, :16],
                                  in_=y3_full[q * CH:(q + 1) * CH, :])

            # ---------------- layer 3 ----------------
            segsum(y3_pad[:, :])
            for b in range(NB):
                rows = min(P, NP - b * P)
                scaled = wk.tile([128, 16], dt.float32, tag="sc3")
                nc.vector.tensor_scalar_mul(out=scaled[:], in0=aggS[:, b * D:b * D + 16],
                                            scalar1=dgv[:, b:b + 1])
                ps_r = ps_a.tile([128, 16], dt.float32, tag="ps_y3")
                nc.tensor.matmul(ps_r[:], lhsT=h2T[:, b * P:(b + 1) * P], rhs=Wr2p[:],
                                 start=True, stop=True)
                lg = wk.tile([128, 16], dt.float32, tag="lg")
                nc.vector.tensor_add(out=lg[:], in0=scaled[:], in1=ps_r[:])
                nc.vector.tensor_add(out=lg[:], in0=lg[:], in1=B2[:])
                mx = wk.tile([128, 1], dt.float32, tag="mx")
                nc.vector.tensor_reduce(out=mx[:], in_=lg[:, :10],
                                        axis=mybir.AxisListType.X, op=mybir.AluOpType.max)
                nmx = wk.tile([128, 1], dt.float32, tag="nmx")
                nc.vector.tensor_scalar_mul(out=nmx[:], in0=mx[:], scalar1=-1.0)
                ex = wk.tile([128, 10], dt.float32, tag="ex")
                nc.scalar.activation(out=ex[:], in_=lg[:, :10],
                                     func=mybir.ActivationFunctionType.Exp, bias=nmx[:])
                sm = wk.tile([128, 1], dt.float32, tag="sm")
                nc.vector.tensor_reduce(out=sm[:], in_=ex[:],
                                        axis=mybir.AxisListType.X, op=mybir.AluOpType.add)
                ls = wk.tile([128, 1], dt.float32, tag="ls")
                nc.scalar.activation(out=ls[:], in_=sm[:],
                                     func=mybir.ActivationFunctionType.Ln)
                sh = wk.tile([128, 1], dt.float32, tag="sh")
                nc.vector.tensor_add(out=sh[:], in0=mx[:], in1=ls[:])
                r = wk.tile([128, 10], dt.float32, tag="r")
                nc.vector.tensor_scalar_sub(out=r[:], in0=lg[:, :10], scalar1=sh[:])
                nc.sync.dma_start(out=out_ext[b * P:b * P + rows, :], in_=r[:rows, :])
    nc.compile()
    return nc


def _get_runner(edge_index):
    key = hash(np.asarray(edge_index).tobytes())
    if _STATE.get("key") == key:
        return _STATE["runner"], _STATE["meta"]
    meta = _build_meta(edge_index)
    nc = _build_program(meta)
    runner = _SpmdRunner(nc, NCORES)
    _STATE.update(key=key, runner=runner, meta=meta)
    return runner, meta


def _in_maps(meta, x, W_out0, b0, W_root0, W_out1, b1, W_root1, W_out2, b2, W_root2):
    x = np.asarray(x, np.float32)
    w2p = np.zeros((D, 16), np.float32); w2p[:, :10] = np.asarray(W_out2)
    wr2p = np.zeros((D, 16), np.float32); wr2p[:, :10] = np.asarray(W_root2)
    b2r = np.zeros((128, 16), np.float32); b2r[:, :10] = np.asarray(b2)[None, :]
    maps = []
    for p in range(NCORES):
        xT = np.zeros((D, NPAD), np.float32)
        xT[:, :NP] = x[p * NP:(p + 1) * NP].T
        maps.append(dict(
            x_full=x, xT=xT, gidx=meta["gidx_w"][p], dsto=meta["dstoff"][p],
            dgin=meta["deginv"][p],
            w0=np.asarray(W_out0, np.float32), wr0=np.asarray(W_root0, np.float32),
            w1=np.asarray(W_out1, np.float32), wr1=np.asarray(W_root1, np.float32),
            w2p=w2p, wr2p=wr2p,
            b0=np.asarray(b0, np.float32).reshape(D, 1),
            b1=np.asarray(b1, np.float32).reshape(D, 1),
            b2=b2r))
    return maps


def kernel(**inputs):
    edge_index = np.asarray(inputs["edge_index"])
    runner, meta = _get_runner(edge_index)
    maps = _in_maps(meta, inputs["x"], inputs["W_out0"], inputs["b0"],
                    inputs["W_root0"], inputs["W_out1"], inputs["b1"],
                    inputs["W_root1"], inputs["W_out2"], inputs["b2"],
                    inputs["W_root2"])
    res, _ = runner.run(maps, repeats=1)
    return np.concatenate([res[p]["out"] for p in range(NCORES)], axis=0)


def kernel_timed(inputs, repeats=12):
    edge_index = np.asarray(inputs["edge_index"])
    runner, meta = _get_runner(edge_index)
    maps = _in_maps(meta, inputs["x"], inputs["W_out0"], inputs["b0"],
                    inputs["W_root0"], inputs["W_out1"], inputs["b1"],
                    inputs["W_root1"], inputs["W_out2"], inputs["b2"],
                    inputs["W_root2"])
    res, times = runner.run(maps, repeats=repeats)
    out = np.concatenate([res[p]["out"] for p in range(NCORES)], axis=0)
    return out, times


# revision 10
# speedup vs baseline: 1.2959x; 1.2959x over previous
"""ClusterGCNConvNet on 8 TRN2 NeuronCores.

Sharding: nodes partitioned into 8 contiguous ranges (12500/core). Each core
aggregates over the edges whose dst lands in its range, gathering source rows
with dma_gather (int16 indices -> 4 source chunks of 25000 rows). Layer
boundaries exchange the transformed activations y = h @ W_out with an
AllGather, so the per-layer gather reads from a replicated full-size buffer.

Per (dst-block of 128, chunk) the edge segment is padded to a multiple of 128
(same padded length on every core so all 8 cores share one Bass program);
segment-sum is a one-hot matmul: S[e, d] = (dstoff[e] == d) built with an
iota/is_equal, PSUM-accumulated over the segment, then added into an SBUF
accumulator. Transforms run in transposed space (h^T resident in SBUF).
"""
import sys, time
sys.path.insert(0, "/opt/trn_rl_repo")
import numpy as np

N, E = 100_000, 1_600_000
D = 64
NCORES = 8
NP = N // NCORES          # 12500 nodes per core
P = 128
NB = (NP + P - 1) // P    # 98 dst blocks per core
NPAD = NB * P             # 12544
CH = 25_000               # gather-source chunk rows (int16 index range)
NCH = 4
NGRP = 14                 # block groups per chunk pass
GBS = NB // NGRP          # 14 blocks per group

_STATE = {}


class _SpmdRunner:
    """Execute a prebuilt Bass module on 8 cores via PJRT (jit once, run many)."""

    def __init__(self, nc, n_cores):
        import jax
        import jax.numpy as jnp
        from jax.experimental.shard_map import shard_map
        from jax.sharding import Mesh, PartitionSpec
        import concourse.mybir as mybir
        from concourse.bass2jax import (_bass_exec_p, install_neuronx_cc_hook,
                                        partition_id_tensor)
        install_neuronx_cc_hook()
        self.jax, self.nc, self.n_cores = jax, nc, n_cores
        partition_name = nc.partition_id_tensor.name if nc.partition_id_tensor else None
        in_names, out_names, out_avals, zero_outs = [], [], [], []
        for alloc in nc.m.functions[0].allocations:
            if not isinstance(alloc, mybir.MemoryLocationSet):
                continue
            name = alloc.memorylocations[0].name
            if alloc.kind == "ExternalInput":
                if name != partition_name and (nc.dbg_addr is None or name != nc.dbg_addr.name):
                    in_names.append(name)
            elif alloc.kind == "ExternalOutput":
                shape = tuple(alloc.tensor_shape)
                dtype = mybir.dt.np(alloc.dtype)
                out_names.append(name)
                out_avals.append(jax.core.ShapedArray(shape, dtype))
                zero_outs.append(np.zeros(shape, dtype))
        self.in_names, self.out_names = in_names, out_names
        self.out_avals, self.zero_outs = out_avals, zero_outs
        n_params, n_outs = len(in_names), len(out_names)
        all_in_names = list(in_names) + list(out_names)
        dbg_name = nc.dbg_addr.name if nc.dbg_addr is not None else None
        if dbg_name is not None:
            all_in_names.append(dbg_name)
        if partition_name is not None:
            all_in_names.append(partition_name)

        def _body(*args):
            operands = list(args)
            if dbg_name is not None:
                operands.append(jnp.zeros((1, 2), jnp.uint32))
            if partition_name is not None:
                operands.append(partition_id_tensor())
            return tuple(_bass_exec_p.bind(
                *operands, out_avals=tuple(out_avals),
                in_names=tuple(all_in_names), out_names=tuple(out_names),
                lowering_input_output_aliases=(),
                sim_require_finite=False, sim_require_nnan=False, nc=nc))

        devices = jax.devices()[:n_cores]
        self.mesh = Mesh(np.asarray(devices), ("core",))
        self.PartitionSpec = PartitionSpec
        in_specs = (PartitionSpec("core"),) * (n_params + n_outs)
        out_specs = (PartitionSpec("core"),) * n_outs
        donate = tuple(range(n_params, n_params + n_outs))
        self.jitted = jax.jit(
            shard_map(_body, mesh=self.mesh, in_specs=in_specs,
                      out_specs=out_specs, check_rep=False),
            donate_argnums=donate, keep_unused=True)

    def run(self, in_maps, repeats=1):
        jax, n = self.jax, self.n_cores
        from jax.sharding import NamedSharding
        shard = NamedSharding(self.mesh, self.PartitionSpec("core"))
        concat_in = [
            np.concatenate([np.asarray(in_maps[c][name]) for c in range(n)], axis=0)
            for name in self.in_names]
        dev_in = [jax.device_put(a, shard) for a in concat_in]
        times, out_arrs = [], None
        for _ in range(repeats):
            zeros = [np.zeros((n * z.shape[0], *z.shape[1:]), z.dtype)
                     for z in self.zero_outs]
            dev_zeros = [jax.device_put(a, shard) for a in zeros]
            for a in dev_zeros:
                a.block_until_ready()
            t0 = time.perf_counter()
            out_arrs = self.jitted(*dev_in, *dev_zeros)
            for a in out_arrs:
                a.block_until_ready()
            times.append(time.perf_counter() - t0)
        results = [
            {name: np.asarray(out_arrs[i]).reshape(n, *self.out_avals[i].shape)[c]
             for i, name in enumerate(self.out_names)}
            for c in range(n)]
        return results, times


def _build_meta(edge_index):
    src_g = np.asarray(edge_index[0]).astype(np.int64)
    dst_g = np.asarray(edge_index[1]).astype(np.int64)
    deg = np.bincount(dst_g, minlength=N).astype(np.float32)
    deginv_full = (1.0 / np.clip(deg, 1.0, None)).astype(np.float32)

    per_core = []
    counts = np.zeros((NCORES, NCH, NB), np.int64)
    for p in range(NCORES):
        m = (dst_g >= p * NP) & (dst_g < (p + 1) * NP)
        s, d = src_g[m], dst_g[m] - p * NP
        b = d // P
        c = s // CH
        order = np.argsort(c * NB + b, kind="stable")
        s, d, b, c = s[order], d[order], b[order], c[order]
        np.add.at(counts[p], (c, b), 1)
        per_core.append((s, d, b, c))

    ksub = np.maximum(1, -(-counts.max(axis=0) // P))  # [NCH, NB] tiles per run
    # tile layout order: for c, for g, for b in group -> ksub[c][b] tiles
    toff = np.zeros((NCH, NB), np.int64)
    t = 0
    for c in range(NCH):
        for g in range(NGRP):
            for b in range(g * GBS, (g + 1) * GBS):
                toff[c, b] = t
                t += ksub[c, b]
    T_tot = t
    R_tot = T_tot * P

    gidx_w = np.zeros((NCORES, 128, R_tot // 16), np.int16)
    dstoff = np.full((NCORES, 128, T_tot), 255.0, np.float32)
    for p in range(NCORES):
        s, d, b, c = per_core[p]
        flat_idx = np.zeros(R_tot, np.int16)
        flat_off = np.full(R_tot, 255.0, np.float32)
        pos = np.searchsorted(c * NB + b, np.arange(NCH * NB), side="left")
        pos = np.append(pos, len(s))
        for cc in range(NCH):
            for bb in range(NB):
                lo, hi = pos[cc * NB + bb], pos[cc * NB + bb + 1]
                t0 = toff[cc, bb] * P
                n = hi - lo
                flat_idx[t0:t0 + n] = (s[lo:hi] - cc * CH).astype(np.int16)
                flat_off[t0:t0 + n] = (d[lo:hi] % P).astype(np.float32)
        w = flat_idx.reshape(R_tot // 16, 16).T            # [16, R/16]
        gidx_w[p] = np.tile(w, (8, 1))
        dstoff[p] = flat_off.reshape(T_tot, P).T            # [128, T_tot]

    # per-instruction (c,g) row counts and offsets
    n_cg = np.zeros((NCH, NGRP), np.int64)
    for c in range(NCH):
        for g in range(NGRP):
            n_cg[c, g] = ksub[c, g * GBS:(g + 1) * GBS].sum() * P
    ioff = np.zeros((NCH, NGRP), np.int64)   # int16-column offset of instr
    acc = 0
    for c in range(NCH):
        for g in range(NGRP):
            ioff[c, g] = acc // 16
            acc += n_cg[c, g]

    deginv_pad = np.ones((NCORES, 128, NB), np.float32)
    for p in range(NCORES):
        dv = np.ones(NPAD, np.float32)
        dv[:NP] = deginv_full[p * NP:(p + 1) * NP]
        deginv_pad[p] = dv.reshape(NB, P).T
    return dict(ksub=ksub, toff=toff, T_tot=T_tot, R_tot=R_tot, n_cg=n_cg,
                ioff=ioff, gidx_w=gidx_w, dstoff=dstoff, deginv=deginv_pad)


def _build_program(meta):
    import concourse.bacc as bacc
    import concourse.bass as bass
    import concourse.mybir as mybir
    import concourse.tile as tile
    from concourse.masks import make_identity
    dt = mybir.dt
    ksub, toff = meta["ksub"], meta["toff"]
    T_tot, R_tot = meta["T_tot"], meta["R_tot"]
    n_cg, ioff = meta["n_cg"], meta["ioff"]

    nc = bacc.Bacc("TRN2", target_bir_lowering=False, debug=False,
                   num_devices=NCORES)
    x_full = nc.dram_tensor("x_full", [N, D], dt.float32, kind="ExternalInput")
    xT_in = nc.dram_tensor("xT", [D, NPAD], dt.float32, kind="ExternalInput")
    gidx = nc.dram_tensor("gidx", [128, R_tot // 16], dt.int16, kind="ExternalInput")
    dsto = nc.dram_tensor("dsto", [128, T_tot], dt.float32, kind="ExternalInput")
    dgin = nc.dram_tensor("dgin", [128, NB], dt.float32, kind="ExternalInput")
    w0 = nc.dram_tensor("w0", [D, D], dt.float32, kind="ExternalInput")
    wr0 = nc.dram_tensor("wr0", [D, D], dt.float32, kind="ExternalInput")
    w1 = nc.dram_tensor("w1", [D, D], dt.float32, kind="ExternalInput")
    wr1 = nc.dram_tensor("wr1", [D, D], dt.float32, kind="ExternalInput")
    w2p = nc.dram_tensor("w2p", [D, 16], dt.float32, kind="ExternalInput")
    wr2p = nc.dram_tensor("wr2p", [D, 16], dt.float32, kind="ExternalInput")
    b0_in = nc.dram_tensor("b0", [D, 1], dt.float32, kind="ExternalInput")
    b1_in = nc.dram_tensor("b1", [D, 1], dt.float32, kind="ExternalInput")
    b2_in = nc.dram_tensor("b2", [128, 16], dt.float32, kind="ExternalInput")
    out_ext = nc.dram_tensor("out", [NP, 10], dt.float32, kind="ExternalOutput")

    y2_full = nc.dram_tensor("y2_full", [N, D], dt.float32, kind="Internal",
                             addr_space="Shared")
    y3_full = nc.dram_tensor("y3_full", [N, 16], dt.float32, kind="Internal",
                             addr_space="Shared")
    y3_pad = nc.dram_tensor("y3_pad", [N, D], dt.float32, kind="Internal")

    with tile.TileContext(nc) as tc:
        with tc.tile_pool(name="res", bufs=1) as res, \
             tc.tile_pool(name="gpool", bufs=2) as gpool, \
             tc.tile_pool(name="spool", bufs=3) as spool, \
             tc.tile_pool(name="wk", bufs=6) as wk, \
             tc.tile_pool(name="ps_a", bufs=2, space="PSUM") as ps_a, \
             tc.tile_pool(name="ps_b", bufs=4, space="PSUM") as ps_b, \
             tc.tile_pool(name="dram", bufs=1, space="DRAM") as dram:

            it = res.tile([128, R_tot // 16], dt.int16)
            nc.sync.dma_start(out=it[:], in_=gidx[:, :])
            doff = res.tile([128, T_tot], dt.float32)
            nc.sync.dma_start(out=doff[:], in_=dsto[:, :])
            dgv = res.tile([128, NB], dt.float32)
            nc.sync.dma_start(out=dgv[:], in_=dgin[:, :])
            W0 = res.tile([D, D], dt.float32); nc.sync.dma_start(out=W0[:], in_=w0[:, :])
            Wr0 = res.tile([D, D], dt.float32); nc.sync.dma_start(out=Wr0[:], in_=wr0[:, :])
            W1 = res.tile([D, D], dt.float32); nc.sync.dma_start(out=W1[:], in_=w1[:, :])
            Wr1 = res.tile([D, D], dt.float32); nc.sync.dma_start(out=Wr1[:], in_=wr1[:, :])
            W2p = res.tile([D, 16], dt.float32); nc.sync.dma_start(out=W2p[:], in_=w2p[:, :])
            Wr2p = res.tile([D, 16], dt.float32); nc.sync.dma_start(out=Wr2p[:], in_=wr2p[:, :])
            B0 = res.tile([D, 1], dt.float32); nc.sync.dma_start(out=B0[:], in_=b0_in[:, :])
            B1 = res.tile([D, 1], dt.float32); nc.sync.dma_start(out=B1[:], in_=b1_in[:, :])
            B2 = res.tile([128, 16], dt.float32); nc.sync.dma_start(out=B2[:], in_=b2_in[:, :])

            ident = res.tile([128, 128], dt.float32)
            make_identity(nc, ident[:])
            iota_i = res.tile([128, 128], dt.int32)
            nc.gpsimd.iota(iota_i[:], pattern=[[1, 128]], base=0, channel_multiplier=0)
            iota_f = res.tile([128, 128], dt.float32)
            nc.vector.tensor_copy(out=iota_f[:], in_=iota_i[:])

            h1T = res.tile([D, NPAD], dt.float32)
            h2T = res.tile([D, NPAD], dt.float32)
            aggS = res.tile([128, NB * D], dt.float32)

            def segsum(src_ap):
                """gather + one-hot segment-sum into aggS (raw sums)."""
                for c in range(NCH):
                    for g in range(NGRP):
                        n = int(n_cg[c, g])
                        nk = n // P
                        gt = gpool.tile([128, nk, D], dt.float32, tag="g")
                        nc.gpsimd.dma_gather(
                            out_ap=gt[:], in_ap=src_ap[c * CH:(c + 1) * CH, :],
                            idxs_ap=it[:, int(ioff[c, g]):int(ioff[c, g]) + n // 16],
                            num_idxs=n, num_idxs_reg=n, elem_size=D,
                            single_packet=False)
                        base_t = int(toff[c, g * GBS])
                        for b in range(g * GBS, (g + 1) * GBS):
                            kk = int(ksub[c, b])
                            t0 = int(toff[c, b])
                            ps = ps_a.tile([128, D], dt.float32, tag="ps_a")
                            for k in range(kk):
                                S = spool.tile([128, 128], dt.float32, tag="S")
                                nc.vector.tensor_tensor(
                                    out=S[:],
                                    in0=doff[:, t0 + k:t0 + k + 1].to_broadcast([128, 128]),
                                    in1=iota_f[:],
                                    op=mybir.AluOpType.is_equal)
                                nc.tensor.matmul(
                                    ps[:], lhsT=S[:], rhs=gt[:, t0 - base_t + k, :],
                                    start=(k == 0), stop=(k == kk - 1))
                            dst = aggS[:, b * D:(b + 1) * D]
                            if c == 0:
                                nc.vector.tensor_copy(out=dst, in_=ps[:])
                            else:
                                nc.vector.tensor_add(out=dst, in0=dst, in1=ps[:])

            # ---------------- layer 1 ----------------
            cc1_in = dram.tile([NP, D], dt.float32)
            segsum(x_full[:, :])
            for b in range(NB):
                rows = min(P, NP - b * P)
                scaled = wk.tile([128, D], dt.float32, tag="scaled")
                nc.vector.tensor_scalar_mul(out=scaled[:], in0=aggS[:, b * D:(b + 1) * D],
                                            scalar1=dgv[:, b:b + 1])
                ps_t = ps_b.tile([D, 128], dt.float32, tag="ps_t")
                nc.tensor.transpose(out=ps_t[:], in_=scaled[:], identity=ident[:])
                aggT = wk.tile([D, 128], dt.float32, tag="aggT")
                nc.vector.tensor_copy(out=aggT[:], in_=ps_t[:])
                xTb = wk.tile([D, 128], dt.float32, tag="xTb")
                nc.sync.dma_start(out=xTb[:], in_=xT_in[:, b * P:(b + 1) * P])
                ps_h = ps_b.tile([D, 128], dt.float32, tag="ps_t")
                nc.tensor.matmul(ps_h[:], lhsT=W0[:], rhs=aggT[:], start=True, stop=False)
                nc.tensor.matmul(ps_h[:], lhsT=Wr0[:], rhs=xTb[:],
                                 start=False, stop=True)
                nc.scalar.activation(out=h1T[:, b * P:(b + 1) * P], in_=ps_h[:],
                                     func=mybir.ActivationFunctionType.Relu, bias=B0[:])
                ps_y = ps_a.tile([128, D], dt.float32, tag="ps_a")
                nc.tensor.matmul(ps_y[:], lhsT=h1T[:, b * P:(b + 1) * P], rhs=W1[:],
                                 start=True, stop=True)
                yrow = wk.tile([128, D], dt.float32, tag="yrow")
                nc.vector.tensor_copy(out=yrow[:], in_=ps_y[:])
                nc.sync.dma_start(out=cc1_in[b * P:b * P + rows, :], in_=yrow[:rows, :])
            nc.gpsimd.collective_compute(
                "AllGather", mybir.AluOpType.bypass,
                replica_groups=[list(range(NCORES))],
                ins=[cc1_in[:].opt()], outs=[y2_full[:, :].opt()])

            # ---------------- layer 2 ----------------
            cc2_in = dram.tile([NP, 16], dt.float32)
            segsum(y2_full[:, :])
            for b in range(NB):
                rows = min(P, NP - b * P)
                scaled = wk.tile([128, D], dt.float32, tag="scaled")
                nc.vector.tensor_scalar_mul(out=scaled[:], in0=aggS[:, b * D:(b + 1) * D],
                                            scalar1=dgv[:, b:b + 1])
                ps_t = ps_b.tile([D, 128], dt.float32, tag="ps_t")
                nc.tensor.transpose(out=ps_t[:], in_=scaled[:], identity=ident[:])
                tT = wk.tile([D, 128], dt.float32, tag="aggT")
                nc.vector.tensor_copy(out=tT[:], in_=ps_t[:])
                ps_r = ps_b.tile([D, 128], dt.float32, tag="ps_t")
                nc.tensor.matmul(ps_r[:], lhsT=Wr1[:], rhs=h1T[:, b * P:(b + 1) * P],
                                 start=True, stop=True)
                hsum = wk.tile([D, 128], dt.float32, tag="hsum")
                nc.vector.tensor_add(out=hsum[:], in0=tT[:], in1=ps_r[:])
                nc.scalar.activation(out=h2T[:, b * P:(b + 1) * P], in_=hsum[:],
                                     func=mybir.ActivationFunctionType.Relu, bias=B1[:])
                ps_y = ps_a.tile([128, 16], dt.float32, tag="ps_y3")
                nc.tensor.matmul(ps_y[:], lhsT=h2T[:, b * P:(b + 1) * P], rhs=W2p[:],
                                 start=True, stop=True)
                yrow = wk.tile([128, 16], dt.float32, tag="yrow3")
                nc.vector.tensor_copy(out=yrow[:], in_=ps_y[:])
                nc.sync.dma_start(out=cc2_in[b * P:b * P + rows, :], in_=yrow[:rows, :])
            nc.gpsimd.collective_compute(
                "AllGather", mybir.AluOpType.bypass,
                replica_groups=[list(range(NCORES))],
                ins=[cc2_in[:].opt()], outs=[y3_full[:, :].opt()])
            # expand [N,16] -> [N,64]-strided so gather rows are 256B apart
            for q in range(NCH):
                nc.sync.dma_start(out=y3_pad[q * CH:(q + 1) * CH, :16],
                                  in_=y3_full[q * CH:(q + 1) * CH, :])

            # ---------------- layer 3 ----------------
            segsum(y3_pad[:, :])
            for b in range(NB):
                rows = min(P, NP - b * P)
                scaled = wk.tile([128, 16], dt.float32, tag="sc3")
                nc.vector.tensor_scalar_mul(out=scaled[:], in0=aggS[:, b * D:b * D + 16],
                                            scalar1=dgv[:, b:b + 1])
                ps_r = ps_a.tile([128, 16], dt.float32, tag="ps_y3")
                nc.tensor.matmul(ps_r[:], lhsT=h2T[:, b * P:(b + 1) * P], rhs=Wr2p[:],
                                 start=True, stop=True)
                lg = wk.tile([128, 16], dt.float32, tag="lg")
                nc.vector.tensor_add(out=lg[:], in0=scaled[:], in1=ps_r[:])
                nc.vector.tensor_add(out=lg[:], in0=lg[:], in1=B2[:])
                mx = wk.tile([128, 1], dt.float32, tag="mx")
                nc.vector.tensor_reduce(out=mx[:], in_=lg[:, :10],
                                        axis=mybir.AxisListType.X, op=mybir.AluOpType.max)
                nmx = wk.tile([128, 1], dt.float32, tag="nmx")
                nc.vector.tensor_scalar_mul(out=nmx[:], in0=mx[:], scalar1=-1.0)
                ex = wk.tile([128, 10], dt.float32, tag="ex")
                nc.scalar.activation(out=ex[:], in_=lg[:, :10],
                                     func=mybir.ActivationFunctionType.Exp, bias=nmx[:])
                sm = wk.tile([128, 1], dt.float32, tag="sm")
                nc.vector.tensor_reduce(out=sm[:], in_=ex[:],
                                        axis=mybir.AxisListType.X, op=mybir.AluOpType.add)
                ls = wk.tile([128, 1], dt.float32, tag="ls")
                nc.scalar.activation(out=ls[:], in_=sm[:],
                                     func=mybir.ActivationFunctionType.Ln)
                sh = wk.tile([128, 1], dt.float32, tag="sh")
                nc.vector.tensor_add(out=sh[:], in0=mx[:], in1=ls[:])
                r = wk.tile([128, 10], dt.float32, tag="r")
                nc.vector.tensor_scalar_sub(out=r[:], in0=lg[:, :10], scalar1=sh[:])
                nc.sync.dma_start(out=out_ext[b * P:b * P + rows, :], in_=r[:rows, :])
    nc.compile()
    return nc


def _get_runner(edge_index):
    key = hash(np.asarray(edge_index).tobytes())
    if _STATE.get("key") == key:
        return _STATE["runner"], _STATE["meta"]
    meta = _build_meta(edge_index)
    nc = _build_program(meta)
    runner = _SpmdRunner(nc, NCORES)
    _STATE.update(key=key, runner=runner, meta=meta)
    return runner, meta


def _in_maps(meta, x, W_out0, b0, W_root0, W_out1, b1, W_root1, W_out2, b2, W_root2):
    x = np.asarray(x, np.float32)
    w2p = np.zeros((D, 16), np.float32); w2p[:, :10] = np.asarray(W_out2)
    wr2p = np.zeros((D, 16), np.float32); wr2p[:, :10] = np.asarray(W_root2)
    b2r = np.zeros((128, 16), np.float32); b2r[:, :10] = np.asarray(b2)[None, :]
    maps = []
    for p in range(NCORES):
        xT = np.zeros((D, NPAD), np.float32)
        xT[:, :NP] = x[p * NP:(p + 1) * NP].T
        maps.append(dict(
            x_full=x, xT=xT, gidx=meta["gidx_w"][p], dsto=meta["dstoff"][p],
            dgin=meta["deginv"][p],
            w0=np.asarray(W_out0, np.float32), wr0=np.asarray(W_root0, np.float32),
            w1=np.asarray(W_out1, np.float32), wr1=np.asarray(W_root1, np.float32),
            w2p=w2p, wr2p=wr2p,
            b0=np.asarray(b0, np.float32).reshape(D, 1),
            b1=np.asarray(b1, np.float32).reshape(D, 1),
            b2=b2r))
    return maps


def kernel(**inputs):
    edge_index = np.asarray(inputs["edge_index"])
    runner, meta = _get_runner(edge_index)
    maps = _in_maps(meta, inputs["x"], inputs["W_out0"], inputs["b0"],
                    inputs["W_root0"], inputs["W_out1"], inputs["b1"],
                    inputs["W_root1"], inputs["W_out2"], inputs["b2"],
                    inputs["W_root2"])
    res, _ = runner.run(maps, repeats=1)
    return np.concatenate([res[p]["out"] for p in range(NCORES)], axis=0)


def kernel_timed(inputs, repeats=12):
    edge_index = np.asarray(inputs["edge_index"])
    runner, meta = _get_runner(edge_index)
    maps = _in_maps(meta, inputs["x"], inputs["W_out0"], inputs["b0"],
                    inputs["W_root0"], inputs["W_out1"], inputs["b1"],
                    inputs["W_root1"], inputs["W_out2"], inputs["b2"],
                    inputs["W_root2"])
    res, times = runner.run(maps, repeats=repeats)
    out = np.concatenate([res[p]["out"] for p in range(NCORES)], axis=0)
    return out, times
